# revision 1
# baseline (speedup 1.0000x reference)
import numpy as np
import concourse.bass as bass
import concourse.bacc as bacc
import concourse.mybir as mybir
from concourse.bass_utils import run_bass_kernel_spmd
from concourse import tile

# DigitCapsules dynamic routing, data-parallel over batch on 8 cores.
# B=512, R=1152, C=10, O=16, I=8; per core Bl=64.
#
# Device layout: partitions p = parity*64 + b  (r = 2*rp + parity), so every
# per-(b,r) routing quantity is partition-local. u_hat for a chunk of r-pairs
# is generated by K=16 matmuls whose stationary operand is a block-diagonal
# "pair canvas" [16, 128] staged on-chip from the compact x-pair input.
# Iteration 1 uses uniform coupling, so s_1 = 0.1 * sum_r u_hat is computed
# as a single dense PE accumulation over all (r, i) with no u_hat storage.

NCORES = 8
B, R, C, O, I = 512, 1152, 10, 16, 8
Bl = B // NCORES          # 64 batch per core
CO = C * O                # 160
NP = R // 2               # 576 r-pairs
PAIRS_PER_CHUNK = 24      # 48 r per chunk; 8 psum banks x 3 pairs
NCHUNK = NP // PAIRS_PER_CHUNK  # 24
FCH = PAIRS_PER_CHUNK * CO      # 3840 free elems per chunk
EPS = 1e-8

_cache = {}


def _build_program():
    if "nc" in _cache:
        return _cache["nc"]
    nc = bacc.Bacc("TRN2", target_bir_lowering=False, debug=False)
    f32 = mybir.dt.float32
    xp_d = nc.dram_tensor("xp", [16, NP, 64], f32, kind="ExternalInput")
    wpair_d = nc.dram_tensor("wpair", [16, NP, CO], f32, kind="ExternalInput")
    NG = NP // 8              # 72 stacked groups of 8 pairs (K=128)
    xps_d = nc.dram_tensor("xps", [128, NG, 64], f32, kind="ExternalInput")
    wps_d = nc.dram_tensor("wps", [128, NG, CO], f32, kind="ExternalInput")
    out_d = nc.dram_tensor("v_out", [Bl, CO], f32, kind="ExternalOutput")

    AX = mybir.AxisListType
    ALU = mybir.AluOpType
    ACTF = mybir.ActivationFunctionType

    def ap(t, dims, offset=0):
        return bass.AP(t.tensor, offset, dims)

    with tile.TileContext(nc) as tc:
        with (
            tc.tile_pool(name="xpp", bufs=3) as xp_pool,
            tc.tile_pool(name="stk", bufs=2) as stk_pool,
            tc.tile_pool(name="cv", bufs=2) as cv_pool,
            tc.tile_pool(name="wp", bufs=2) as wp_pool,
            tc.tile_pool(name="psum", bufs=7, space="PSUM") as psum_pool,
            tc.tile_pool(name="ps1", bufs=1, space="PSUM") as ps1_pool,
            tc.tile_pool(name="uch", bufs=2) as uch_pool,
            tc.tile_pool(name="tmp", bufs=1) as tmp_pool,
            tc.tile_pool(name="res", bufs=1) as res_pool,
        ):
            b_ij = res_pool.tile([128, NP * C], f32, tag="bij")       # logits, (rp, c)
            s_acc = res_pool.tile([128, CO], f32, tag="sacc")
            vtile = res_pool.tile([128, CO], f32, tag="vt")           # v on both halves
            s_fold = res_pool.tile([64, CO], f32, tag="sfold")
            sq = res_pool.tile([64, C], f32, tag="sq")
            f1 = res_pool.tile([64, C], f32, tag="f1")
            f2 = res_pool.tile([64, C], f32, tag="f2")

            nc.vector.memset(b_ij[:], 0.0)

            def dma_xp_chunk(k):
                xpch = xp_pool.tile([16, PAIRS_PER_CHUNK * 64], f32, tag="x")
                nc.sync.dma_start(
                    xpch[:],
                    xp_d[:, k * PAIRS_PER_CHUNK:(k + 1) * PAIRS_PER_CHUNK, :],
                )
                return xpch

            def squash_and_store(t):
                # fold parity halves, squash, broadcast v (or emit output)
                upper = tmp_pool.tile([64, CO], f32, tag="up")
                nc.sync.dma_start(upper[:], s_acc[64:128, :])
                nc.vector.tensor_tensor(s_fold[:], s_acc[0:64, :], upper[:], op=ALU.add)
                if t == 0:
                    nc.vector.tensor_scalar_mul(s_fold[:], s_fold[:], 0.1)
                prod = tmp_pool.tile([64, CO], f32, tag="pr")
                nc.vector.tensor_tensor(prod[:], s_fold[:], s_fold[:], op=ALU.mult)
                nc.vector.tensor_reduce(
                    sq[:], ap(prod, [[CO, 64], [16, C], [1, O]]), axis=AX.X, op=ALU.add
                )
                onep = tmp_pool.tile([64, C], f32, tag="q1")
                nc.vector.tensor_scalar_add(onep[:], sq[:], 1.0)
                nc.vector.reciprocal(f1[:], onep[:])
                rt = tmp_pool.tile([64, C], f32, tag="q2")
                nc.vector.tensor_scalar_add(rt[:], sq[:], EPS)
                nc.scalar.activation(rt[:], rt[:], ACTF.Sqrt)
                nc.vector.reciprocal(f2[:], rt[:])
                nc.vector.tensor_tensor(f1[:], f1[:], f2[:], op=ALU.mult)
                nc.vector.tensor_tensor(f1[:], f1[:], sq[:], op=ALU.mult)
                nc.vector.tensor_tensor(
                    vtile[0:64, :], s_fold[:], ap(f1, [[C, 64], [1, C], [0, O]]),
                    op=ALU.mult,
                )
                if t < 2:
                    nc.sync.dma_start(vtile[64:128, :], vtile[0:64, :])
                else:
                    nc.sync.dma_start(out_d[:], vtile[0:64, :])

            # ---- pass A: s_1 = sum_{r,i} x*W, PE only. 8 r-pairs stacked on
            # the contraction (K=128, host-prepped layout) since s_1 sums all r.
            s1ps = ps1_pool.tile([64, CO], f32, tag="s1")
            GA = 9  # stacked groups per chunk (72 total)
            for k in range(NG // GA):
                xk = stk_pool.tile([128, GA * 64], f32, tag="xk")
                nc.sync.dma_start(xk[:], xps_d[:, k * GA:(k + 1) * GA, :])
                wk = stk_pool.tile([128, GA * CO], f32, tag="wk")
                nc.sync.dma_start(wk[:], wps_d[:, k * GA:(k + 1) * GA, :])
                for g in range(GA):
                    nc.tensor.matmul(
                        s1ps[:],
                        xk[:, g * 64:(g + 1) * 64],
                        wk[:, g * CO:(g + 1) * CO],
                        start=(k == 0 and g == 0),
                        stop=(k == NG // GA - 1 and g == GA - 1),
                    )
            nc.vector.memset(s_acc[:], 0.0)
            nc.vector.tensor_copy(s_acc[0:64, :], s1ps[:])
            squash_and_store(0)

            # ---- passes B (t=1), C (t=2): regen u_hat chunks + routing
            cv_count = 0
            for t in (1, 2):
                nc.vector.memset(s_acc[:], 0.0)
                for k in range(NCHUNK):
                    xpch = dma_xp_chunk(k)
                    wch = wp_pool.tile([16, PAIRS_PER_CHUNK * CO], f32, tag="w")
                    nc.sync.dma_start(
                        wch[:],
                        wpair_d[:, k * PAIRS_PER_CHUNK:(k + 1) * PAIRS_PER_CHUNK, :],
                    )
                    # stage block-diagonal canvases: [16, 24*128]
                    cch = cv_pool.tile([16, PAIRS_PER_CHUNK * 128], f32, tag="c")
                    if cv_count < 2:
                        nc.vector.memset(cch[:], 0.0)
                    cv_count += 1
                    # parity 0 rows 0:8 cols rp*128+0:64 ; parity 1 rows 8:16 cols rp*128+64:128
                    cpitch = PAIRS_PER_CHUNK * 128
                    xpitch = PAIRS_PER_CHUNK * 64
                    nc.sync.dma_start(
                        ap(cch, [[cpitch, 8], [128, PAIRS_PER_CHUNK], [1, 64]], 0),
                        ap(xpch, [[xpitch, 8], [64, PAIRS_PER_CHUNK], [1, 64]], 0),
                    )
                    nc.sync.dma_start(
                        ap(cch, [[cpitch, 8], [128, PAIRS_PER_CHUNK], [1, 64]],
                           8 * cpitch + 64),
                        ap(xpch, [[xpitch, 8], [64, PAIRS_PER_CHUNK], [1, 64]],
                           8 * xpitch),
                    )
                    uch = uch_pool.tile([128, FCH], f32, tag="u")
                    for j in range(8):  # 8 psum tiles, 3 pairs each
                        ps = psum_pool.tile([128, 3 * CO], f32, tag="ps")
                        for q in range(3):
                            rp = j * 3 + q
                            nc.tensor.matmul(
                                ps[:, q * CO:(q + 1) * CO],
                                cch[:, rp * 128:(rp + 1) * 128],
                                wch[:, rp * CO:(rp + 1) * CO],
                                start=True, stop=True,
                            )
                        nc.scalar.copy(
                            uch[:, j * 3 * CO:(j + 1) * 3 * CO], ps[:]
                        )

                    # uch free dims: (rp 24, c 10, o 16); strides rp=160, c=16, o=1
                    # a_{t-1}[p,(rp,c)] = sum_o u*v ; b_ij += a
                    tmp = tmp_pool.tile([128, FCH], f32, tag="m1")
                    nc.gpsimd.tensor_tensor(
                        tmp[:], uch[:],
                        ap(vtile, [[CO, 128], [0, PAIRS_PER_CHUNK], [1, CO]]),
                        op=ALU.mult,
                    )
                    ared = tmp_pool.tile([128, PAIRS_PER_CHUNK * C], f32, tag="ar")
                    nc.vector.tensor_reduce(
                        ared[:],
                        ap(tmp, [[FCH, 128], [CO, PAIRS_PER_CHUNK], [16, C], [1, O]]),
                        axis=AX.X, op=ALU.add,
                    )
                    bsl = b_ij[:, k * PAIRS_PER_CHUNK * C:(k + 1) * PAIRS_PER_CHUNK * C]
                    nc.vector.tensor_tensor(bsl, bsl, ared[:], op=ALU.add)
                    # softmax over c
                    cexp = tmp_pool.tile([128, PAIRS_PER_CHUNK * C], f32, tag="ce")
                    nc.scalar.activation(cexp[:], bsl, ACTF.Exp)
                    csum = tmp_pool.tile([128, PAIRS_PER_CHUNK], f32, tag="cs")
                    nc.vector.tensor_reduce(
                        csum[:],
                        ap(cexp, [[PAIRS_PER_CHUNK * C, 128], [C, PAIRS_PER_CHUNK], [1, C]]),
                        axis=AX.X, op=ALU.add,
                    )
                    crec = tmp_pool.tile([128, PAIRS_PER_CHUNK], f32, tag="cr")
                    nc.vector.reciprocal(crec[:], csum[:])
                    cij = tmp_pool.tile([128, PAIRS_PER_CHUNK * C], f32, tag="cij")
                    nc.vector.tensor_tensor(
                        cij[:], cexp[:],
                        ap(crec, [[PAIRS_PER_CHUNK, 128], [1, PAIRS_PER_CHUNK], [0, C]]),
                        op=ALU.mult,
                    )
                    # s += sum_r cij * u
                    tmp2 = tmp_pool.tile([128, FCH], f32, tag="m2")
                    nc.vector.tensor_tensor(
                        tmp2[:], uch[:],
                        ap(cij, [[PAIRS_PER_CHUNK * C, 128], [C, PAIRS_PER_CHUNK], [1, C], [0, O]]),
                        op=ALU.mult,
                    )
                    sred = tmp_pool.tile([128, CO], f32, tag="sr")
                    nc.vector.tensor_reduce(
                        sred[:],
                        ap(tmp2, [[FCH, 128], [16, C], [1, O], [CO, PAIRS_PER_CHUNK]]),
                        axis=AX.X, op=ALU.add,
                    )
                    nc.vector.tensor_tensor(s_acc[:], s_acc[:], sred[:], op=ALU.add)
                squash_and_store(t)
    nc.compile()
    _cache["nc"] = nc
    return nc


def _host_prep(x, W):
    # x [B,R,I], W [1,R,C,O,I] -> xp [16, NP, 64] per core, wpair [16, NP, CO]
    Wr = np.ascontiguousarray(
        W[0].reshape(R, CO, I).transpose(2, 0, 1), dtype=np.float32
    )  # [I, R, CO]
    wpair = np.empty((16, NP, CO), np.float32)
    wpair[0:8] = Wr[:, 0::2, :]
    wpair[8:16] = Wr[:, 1::2, :]
    wps = np.ascontiguousarray(
        wpair.reshape(16, NP // 8, 8, CO).transpose(2, 0, 1, 3).reshape(128, NP // 8, CO)
    )
    maps = []
    for core in range(NCORES):
        xl = x[core * Bl:(core + 1) * Bl]          # [64, R, I]
        xp = np.empty((16, NP, 64), np.float32)
        xp[0:8] = xl[:, 0::2, :].transpose(2, 1, 0)
        xp[8:16] = xl[:, 1::2, :].transpose(2, 1, 0)
        xs = np.ascontiguousarray(
            xp.reshape(16, NP // 8, 8, 64).transpose(2, 0, 1, 3).reshape(128, NP // 8, 64)
        )
        maps.append({"xp": xp, "wpair": wpair, "xps": xs, "wps": wps})
    return maps


def kernel(x, W):
    x = np.asarray(x, dtype=np.float32)
    W = np.asarray(W, dtype=np.float32)
    nc = _build_program()
    in_maps = _host_prep(x, W)
    res = run_bass_kernel_spmd(nc, in_maps, list(range(NCORES))).results
    out = np.concatenate([r["v_out"] for r in res], axis=0)  # [B, CO]
    return out.reshape(B, C, O)



# revision 8
# speedup vs baseline: 1.2590x; 1.2590x over previous
import numpy as np
import concourse.bass as bass
import concourse.bacc as bacc
import concourse.mybir as mybir
from concourse.bass_utils import run_bass_kernel_spmd
from concourse import tile

# DigitCapsules dynamic routing, data-parallel over batch on 8 cores.
# B=512, R=1152, C=10, O=16, I=8; per core Bl=64.
#
# Device layout: partitions p = parity*64 + b (r = 2*rp + parity), so every
# per-(b,r) routing quantity is partition-local. Inputs are bf16 and minimal:
# x as xp [16, NP, 64] (i-parity on partitions, (pair, batch) free) and W as
# wp [16, NP, CO]. u_hat for a pair is generated by two K=8 matmuls (one per
# parity, separated by partition offsets), so no block-diagonal canvas
# staging is needed. Iteration 1 uses uniform coupling, so s_1 = 0.1 *
# sum_r u_hat comes from K=16 matmuls (parities summed by the contraction)
# accumulated into a single PSUM bank with no u_hat storage. The big
# elementwise/reduce ops are split half/half across the Vector and GpSimd
# engines; u_hat is held in bf16 to halve their cost.

NCORES = 8
B, R, C, O, I = 512, 1152, 10, 16, 8
Bl = B // NCORES          # 64 batch per core
CO = C * O                # 160
NP = R // 2               # 576 r-pairs
PAIRS_PER_CHUNK = 24      # 48 r per chunk; 8 psum banks x 3 pairs
NCHUNK = NP // PAIRS_PER_CHUNK  # 24
FCH = PAIRS_PER_CHUNK * CO      # 3840 free elems per chunk
HP = PAIRS_PER_CHUNK // 2       # 12 pairs per engine half
EPS = 1e-8

_cache = {}


def _build_program():
    if "nc" in _cache:
        return _cache["nc"]
    nc = bacc.Bacc("TRN2", target_bir_lowering=False, debug=False)
    f32 = mybir.dt.float32
    bf16 = mybir.dt.bfloat16
    xp_d = nc.dram_tensor("xp", [16, NP, 64], bf16, kind="ExternalInput")
    wp_d = nc.dram_tensor("wp", [16, NP, CO], bf16, kind="ExternalInput")
    out_d = nc.dram_tensor("v_out", [Bl, CO], bf16, kind="ExternalOutput")

    AX = mybir.AxisListType
    ALU = mybir.AluOpType
    ACTF = mybir.ActivationFunctionType

    def ap(t, dims, offset=0):
        return bass.AP(t.tensor, offset, dims)

    with tile.TileContext(nc) as tc:
        with (
            tc.tile_pool(name="xch", bufs=3) as xch_pool,
            tc.tile_pool(name="wch", bufs=3) as wch_pool,
            tc.tile_pool(name="psA", bufs=1, space="PSUM") as psA_pool,
            tc.tile_pool(name="psum", bufs=7, space="PSUM") as psum_pool,
            tc.tile_pool(name="uch", bufs=2) as uch_pool,
            tc.tile_pool(name="tmp", bufs=1) as tmp_pool,
            tc.tile_pool(name="res", bufs=1) as res_pool,
        ):
            b_ij = res_pool.tile([128, NP * C], f32, tag="bij")       # (rp, c)
            s_acc = res_pool.tile([128, CO], f32, tag="sacc")
            vtile = res_pool.tile([128, CO], bf16, tag="vt")          # v on both halves
            s_fold = res_pool.tile([64, CO], f32, tag="sfold")
            sq = res_pool.tile([64, C], f32, tag="sq")
            f1 = res_pool.tile([64, C], f32, tag="f1")
            f2 = res_pool.tile([64, C], f32, tag="f2")

            nc.vector.memset(b_ij[:], 0.0)

            def stream_chunk(k):
                # parity-split x and W chunk tiles, all at base partition 0
                # (the PE requires operand base partitions in {0, 32, 64})
                sl = slice(k * PAIRS_PER_CHUNK, (k + 1) * PAIRS_PER_CHUNK)
                xc0 = xch_pool.tile([8, PAIRS_PER_CHUNK * 64], bf16, tag="x0")
                nc.sync.dma_start(xc0[:], xp_d[0:8, sl, :])
                xc1 = xch_pool.tile([8, PAIRS_PER_CHUNK * 64], bf16, tag="x1")
                nc.sync.dma_start(xc1[:], xp_d[8:16, sl, :])
                wc0 = wch_pool.tile([8, PAIRS_PER_CHUNK * CO], bf16, tag="w0")
                nc.sync.dma_start(wc0[:], wp_d[0:8, sl, :])
                wc1 = wch_pool.tile([8, PAIRS_PER_CHUNK * CO], bf16, tag="w1")
                nc.sync.dma_start(wc1[:], wp_d[8:16, sl, :])
                return xc0, xc1, wc0, wc1

            def squash_and_store(t, s_src):
                # s_src: [64, CO] f32 view of folded s; squash, broadcast v
                prod = tmp_pool.tile([64, CO], f32, tag="pr")
                nc.vector.tensor_tensor(prod[:], s_src, s_src, op=ALU.mult)
                nc.vector.tensor_reduce(
                    sq[:], ap(prod, [[CO, 64], [16, C], [1, O]]), axis=AX.X, op=ALU.add
                )
                onep = tmp_pool.tile([64, C], f32, tag="q1")
                nc.vector.tensor_scalar_add(onep[:], sq[:], 1.0)
                nc.vector.reciprocal(f1[:], onep[:])
                rt = tmp_pool.tile([64, C], f32, tag="q2")
                nc.vector.tensor_scalar_add(rt[:], sq[:], EPS)
                nc.scalar.activation(rt[:], rt[:], ACTF.Sqrt)
                nc.vector.reciprocal(f2[:], rt[:])
                nc.vector.tensor_tensor(f1[:], f1[:], f2[:], op=ALU.mult)
                nc.vector.tensor_tensor(f1[:], f1[:], sq[:], op=ALU.mult)
                nc.vector.tensor_tensor(
                    vtile[0:64, :], s_src, ap(f1, [[C, 64], [1, C], [0, O]]),
                    op=ALU.mult,
                )
                if t < 2:
                    nc.sync.dma_start(vtile[64:128, :], vtile[0:64, :])
                else:
                    nc.sync.dma_start(out_d[:], vtile[0:64, :])

            # ---- pass A: s_1 = 0.1 * sum_{r,i} x*W. Accumulate all pairs
            # and both parities into one PSUM bank via K=8 matmuls.
            s1ps = psA_pool.tile([64, CO], f32, tag="s1")
            for k in range(NCHUNK):
                xc0, xc1, wc0, wc1 = stream_chunk(k)
                for q in range(PAIRS_PER_CHUNK):
                    rp = k * PAIRS_PER_CHUNK + q
                    nc.tensor.matmul(
                        s1ps[:],
                        xc0[:, q * 64:(q + 1) * 64],
                        wc0[:, q * CO:(q + 1) * CO],
                        start=(rp == 0),
                        stop=False,
                    )
                    nc.tensor.matmul(
                        s1ps[:],
                        xc1[:, q * 64:(q + 1) * 64],
                        wc1[:, q * CO:(q + 1) * CO],
                        start=False,
                        stop=(rp == NP - 1),
                    )
            nc.vector.tensor_scalar_mul(s_fold[:], s1ps[:], 0.1)
            squash_and_store(0, s_fold[:])

            # ---- passes B (t=1), C (t=2): regen u_hat chunks + routing
            for t in (1, 2):
                nc.vector.memset(s_acc[:], 0.0)
                for k in range(NCHUNK):
                    xc0, xc1, wc0, wc1 = stream_chunk(k)
                    uch = uch_pool.tile([128, FCH], bf16, tag="u")
                    for j in range(8):  # 8 psum tiles, 3 pairs each
                        ps = psum_pool.tile([128, 3 * CO], f32, tag="ps")
                        for q in range(3):
                            p = j * 3 + q
                            nc.tensor.matmul(
                                ps[0:64, q * CO:(q + 1) * CO],
                                xc0[:, p * 64:(p + 1) * 64],
                                wc0[:, p * CO:(p + 1) * CO],
                                start=True, stop=True,
                            )
                            nc.tensor.matmul(
                                ps[64:128, q * CO:(q + 1) * CO],
                                xc1[:, p * 64:(p + 1) * 64],
                                wc1[:, p * CO:(p + 1) * CO],
                                start=True, stop=True,
                            )
                        nc.scalar.copy(
                            uch[:, j * 3 * CO:(j + 1) * 3 * CO], ps[:]
                        )

                    # uch free dims: (rp 24, c 10, o 16); strides rp=160, c=16, o=1
                    # prod = u * v ; ared[p,(rp,c)] = sum_o prod
                    prod = tmp_pool.tile([128, FCH], bf16, tag="m1")
                    nc.vector.tensor_tensor(
                        prod[:, 0:HP * CO], uch[:, 0:HP * CO],
                        ap(vtile, [[CO, 128], [0, HP], [1, CO]]),
                        op=ALU.mult,
                    )
                    nc.gpsimd.tensor_tensor(
                        prod[:, HP * CO:FCH], uch[:, HP * CO:FCH],
                        ap(vtile, [[CO, 128], [0, HP], [1, CO]]),
                        op=ALU.mult,
                    )
                    ared = tmp_pool.tile([128, PAIRS_PER_CHUNK * C], f32, tag="ar")
                    nc.vector.tensor_reduce(
                        ared[:],
                        ap(prod, [[FCH, 128], [CO, PAIRS_PER_CHUNK], [16, C], [1, O]]),
                        axis=AX.X, op=ALU.add,
                    )
                    bsl = b_ij[:, k * PAIRS_PER_CHUNK * C:(k + 1) * PAIRS_PER_CHUNK * C]
                    nc.vector.tensor_tensor(bsl, bsl, ared[:], op=ALU.add)
                    # softmax over c
                    cexp = tmp_pool.tile([128, PAIRS_PER_CHUNK * C], f32, tag="ce")
                    nc.scalar.activation(cexp[:], bsl, ACTF.Exp)
                    csum = tmp_pool.tile([128, PAIRS_PER_CHUNK], f32, tag="cs")
                    nc.vector.tensor_reduce(
                        csum[:],
                        ap(cexp, [[PAIRS_PER_CHUNK * C, 128], [C, PAIRS_PER_CHUNK], [1, C]]),
                        axis=AX.X, op=ALU.add,
                    )
                    crec = tmp_pool.tile([128, PAIRS_PER_CHUNK], f32, tag="cr")
                    nc.vector.reciprocal(crec[:], csum[:])
                    cij = tmp_pool.tile([128, PAIRS_PER_CHUNK * C], bf16, tag="cij")
                    nc.vector.tensor_tensor(
                        cij[:], cexp[:],
                        ap(crec, [[PAIRS_PER_CHUNK, 128], [1, PAIRS_PER_CHUNK], [0, C]]),
                        op=ALU.mult,
                    )
                    # s += sum_rp cij * u
                    tmp2 = tmp_pool.tile([128, FCH], bf16, tag="m2")
                    nc.vector.tensor_tensor(
                        tmp2[:, 0:HP * CO], uch[:, 0:HP * CO],
                        ap(cij, [[PAIRS_PER_CHUNK * C, 128], [C, HP], [1, C], [0, O]]),
                        op=ALU.mult,
                    )
                    nc.gpsimd.tensor_tensor(
                        tmp2[:, HP * CO:FCH], uch[:, HP * CO:FCH],
                        ap(cij, [[PAIRS_PER_CHUNK * C, 128], [C, HP], [1, C], [0, O]], HP * C),
                        op=ALU.mult,
                    )
                    sred = tmp_pool.tile([128, CO], f32, tag="sr")
                    nc.vector.tensor_reduce(
                        sred[:],
                        ap(tmp2, [[FCH, 128], [16, C], [1, O], [CO, PAIRS_PER_CHUNK]]),
                        axis=AX.X, op=ALU.add,
                    )
                    nc.vector.tensor_tensor(s_acc[:], s_acc[:], sred[:], op=ALU.add)
                # fold parity halves, squash
                upper = tmp_pool.tile([64, CO], f32, tag="up")
                nc.sync.dma_start(upper[:], s_acc[64:128, :])
                nc.vector.tensor_tensor(s_fold[:], s_acc[0:64, :], upper[:], op=ALU.add)
                squash_and_store(t, s_fold[:])
    nc.compile()
    _cache["nc"] = nc
    return nc


def _host_prep(x, W):
    # x [B,R,I] f32, W [1,R,C,O,I] f32 -> per-core xp bf16 [16, NP, 64],
    # shared wp bf16 [16, NP, CO]
    import ml_dtypes
    bf = ml_dtypes.bfloat16
    Wr = np.ascontiguousarray(
        W[0].reshape(R, CO, I).transpose(2, 0, 1), dtype=np.float32
    )  # [I, R, CO]
    wpair = np.empty((16, NP, CO), np.float32)
    wpair[0:8] = Wr[:, 0::2, :]
    wpair[8:16] = Wr[:, 1::2, :]
    wpair = wpair.astype(bf)
    maps = []
    for core in range(NCORES):
        xl = x[core * Bl:(core + 1) * Bl]          # [64, R, I]
        xp = np.empty((16, NP, 64), np.float32)
        xp[0:8] = xl[:, 0::2, :].transpose(2, 1, 0)
        xp[8:16] = xl[:, 1::2, :].transpose(2, 1, 0)
        maps.append({"xp": xp.astype(bf), "wp": wpair})
    return maps


def kernel(x, W):
    x = np.asarray(x, dtype=np.float32)
    W = np.asarray(W, dtype=np.float32)
    nc = _build_program()
    in_maps = _host_prep(x, W)
    res = run_bass_kernel_spmd(nc, in_maps, list(range(NCORES))).results
    out = np.concatenate([r["v_out"] for r in res], axis=0)  # [B, CO] bf16
    return out.astype(np.float32).reshape(B, C, O)
